# revision 6
# baseline (speedup 1.0000x reference)
"""Multi-head attention (B=4, T=2048, C=1024, H=16, D=64) on 8 TRN2 NeuronCores.

Sharding: data-parallel over the 4 batches x tensor-parallel over 2 head
groups (8 heads each).  Core c handles batch (c % 4), head group (c // 4).

Per-core kernel (all matmuls in bf16, fp32 accumulation):
  qT = (Wq_g x_b^T + bq_g)        [512, 2048]  (c_out on partitions)
  kT = (Wk_g x_b^T + bk_g)        [512, 2048]
  v  = (x_b Wv_g^T)               [2048, 512]  (t on partitions; bv folded on host)
  per head h:  S^T = kT_h^T-contraction: scoresT[tk, tq] (K=64 matmuls)
               P = exp(S^T / 8)   (scalar engine, PSUM -> SBUF bf16)
               A^T[d, tq] = sum_tk [V_h | 1] P  (M=65: row 64 = softmax sums)
               A_h = A^T[0:64] * (1/sums)  (PE broadcast + DVE mul)
  oT_partial = Wo_g^T-contraction over the 8 heads  [1024, 2048] fp32 -> HBM

Host: out[b] = (oT(b, g0) + oT(b, g1)).T + bo + Wo @ bv
(the V-bias contributes exactly Wo @ bv per row because softmax rows sum to 1).
"""

import sys

if "/opt/trn_rl_repo" not in sys.path:
    sys.path.insert(0, "/opt/trn_rl_repo")

import numpy as np
import ml_dtypes

from concourse.bacc import Bacc
import concourse.mybir as mybir
import concourse.tile as tile
from concourse.bass_utils import run_bass_kernel_spmd

F32 = mybir.dt.float32
F32R = mybir.dt.float32r
BF16 = mybir.dt.bfloat16
EXPF = mybir.ActivationFunctionType.Exp

B, T, C = 4, 2048, 1024
H, D = 16, 64
HPC = 8          # heads per core
CS = HPC * D     # c_out slice per core = 512
NKT = T // 128   # 16 k-tiles over t_k
NQC = T // 512   # 4 q-chunks of 512
P_BUFS = 16


def build_nc():
    nc = Bacc(trn_type="TRN2")
    xT_d = nc.dram_tensor("xT", [C, T], BF16, kind="ExternalInput")
    wq_d = nc.dram_tensor("wqT", [C, CS], BF16, kind="ExternalInput")
    wk_d = nc.dram_tensor("wkT", [C, CS], BF16, kind="ExternalInput")
    wv_d = nc.dram_tensor("wvT", [C, CS], BF16, kind="ExternalInput")
    wo_d = nc.dram_tensor("woT", [CS, C], BF16, kind="ExternalInput")
    bq_d = nc.dram_tensor("bq", [CS, 1], F32, kind="ExternalInput")
    bk_d = nc.dram_tensor("bk", [CS, 1], F32, kind="ExternalInput")
    oT_d = nc.dram_tensor("oT", [C, T], F32, kind="ExternalOutput")

    with tile.TileContext(nc) as tc:
        with (
            tc.tile_pool(name="consts", bufs=1) as consts,
            tc.tile_pool(name="qkv", bufs=1) as qkv,
            tc.tile_pool(name="ptiles", bufs=P_BUFS) as ppool,
            tc.tile_pool(name="small", bufs=2) as small,
            tc.tile_pool(name="ostage", bufs=4) as ostage,
            tc.tile_pool(name="ps", bufs=2, space="PSUM") as ps,
        ):
            # ---- load inputs ----
            xT_sb = consts.tile([128, 8, T], BF16)
            nc.sync.dma_start(out=xT_sb, in_=xT_d[:, :].rearrange("(c p) n -> p c n", p=128))
            wq_sb = consts.tile([128, 8, CS], BF16)
            nc.sync.dma_start(out=wq_sb, in_=wq_d[:, :].rearrange("(c p) n -> p c n", p=128))
            wk_sb = consts.tile([128, 8, CS], BF16)
            nc.sync.dma_start(out=wk_sb, in_=wk_d[:, :].rearrange("(c p) n -> p c n", p=128))
            wv_sb = consts.tile([128, 8, CS], BF16)
            nc.sync.dma_start(out=wv_sb, in_=wv_d[:, :].rearrange("(c p) n -> p c n", p=128))
            wo_sb = consts.tile([128, 4, C], BF16)
            nc.sync.dma_start(out=wo_sb, in_=wo_d[:, :].rearrange("(c p) n -> p c n", p=128))
            bq_sb = consts.tile([128, 4], F32)
            nc.sync.dma_start(out=bq_sb, in_=bq_d[:, :].rearrange("(c p) n -> p (c n)", p=128))
            bk_sb = consts.tile([128, 4], F32)
            nc.sync.dma_start(out=bk_sb, in_=bk_d[:, :].rearrange("(c p) n -> p (c n)", p=128))
            ones_sb = consts.tile([128, 64], F32)
            nc.vector.memset(ones_sb, 1.0)

            qT_sb = qkv.tile([128, 4, T], BF16)
            kT_sb = qkv.tile([128, 4, T], BF16)
            # V per head with a ones column appended: [tk partition, ktile, head, 64+1]
            vh_sb = qkv.tile([128, NKT, HPC, D + 1], BF16)
            nc.vector.memset(vh_sb[:, :, :, D:D + 1], 1.0)
            a_sb = qkv.tile([128, 4, T], BF16)

            # ---- QKV projections ----
            # q/k: out[c_out, t] ; lhsT = W^T tile [c_in 128, c_out 128], rhs = xT
            for (w_sb, b_sb, dst) in ((wq_sb, bq_sb, qT_sb), (wk_sb, bk_sb, kT_sb)):
                for mt in range(4):
                    for t in range(4):
                        pmm = ps.tile([128, 512], F32, tag="mm", name="pmm")
                        for ci in range(8):
                            nc.tensor.matmul(
                                pmm,
                                w_sb[:, ci, mt * 128:(mt + 1) * 128],
                                xT_sb[:, ci, t * 512:(t + 1) * 512],
                                start=(ci == 0), stop=(ci == 7),
                            )
                        nc.vector.tensor_scalar_add(
                            dst[:, mt, t * 512:(t + 1) * 512], pmm, b_sb[:, mt:mt + 1]
                        )
            # v: out[t, c_out]; lhsT = xT tile [c_in 128, t 128], rhs = wv tile
            for tt in range(NKT):
                pmm = ps.tile([128, 512], F32, tag="mm", name="pmm")
                for ci in range(8):
                    nc.tensor.matmul(
                        pmm,
                        xT_sb[:, ci, tt * 128:(tt + 1) * 128],
                        wv_sb[:, ci, :],
                        start=(ci == 0), stop=(ci == 7),
                    )
                for h in range(HPC):
                    nc.vector.tensor_copy(
                        vh_sb[:, tt, h, 0:D], pmm[:, h * D:(h + 1) * D]
                    )

            # ---- attention per head ----
            for h in range(HPC):
                hp, off = h // 2, 64 * (h % 2)
                ptiles = []
                for kt in range(NKT):
                    pt = ppool.tile([128, T], BF16, tag="P", bufs=P_BUFS, name="pt")
                    ptiles.append(pt)
                    for half in range(2):
                        sc = ps.tile([128, 1024], F32, tag="sc", name="sc")
                        for j in range(2):
                            q0 = half * 1024 + j * 512
                            nc.tensor.matmul(
                                sc[:, j * 512:(j + 1) * 512],
                                kT_sb[off:off + 64, hp, kt * 128:(kt + 1) * 128],
                                qT_sb[off:off + 64, hp, q0:q0 + 512],
                                start=True, stop=True,
                            )
                        nc.scalar.activation(
                            pt[:, half * 1024:(half + 1) * 1024], sc, EXPF, scale=0.125
                        )
                for qc in range(NQC):
                    av = ps.tile([65, 512], F32, tag="av", name="av")
                    for kt in range(NKT):
                        nc.tensor.matmul(
                            av,
                            vh_sb[:, kt, h, :],
                            ptiles[kt][:, qc * 512:(qc + 1) * 512],
                            start=(kt == 0), stop=(kt == NKT - 1),
                        )
                    ssum = small.tile([1, 512], F32, tag="ssum", bufs=1, name="ssum")
                    nc.vector.tensor_copy(ssum, av[64:65, :])
                    rec = small.tile([1, 512], F32, tag="rec", bufs=1, name="rec")
                    nc.vector.reciprocal_approx_fast(out=rec, in_=ssum)
                    rbs = small.tile([64, 512], F32, tag="rbs", bufs=1, name="rbs")
                    nc.gpsimd.partition_broadcast(rbs, rec)
                    nc.vector.tensor_mul(
                        a_sb[off:off + 64, hp, qc * 512:(qc + 1) * 512],
                        av[0:64, :], rbs,
                    )

            # ---- output projection: oT[c_out, t] = WoT.T-contraction @ A ----
            for qc in range(NQC):
                for mt in range(8):
                    pmm = ps.tile([128, 512], F32, tag="mm", name="pmm")
                    for ci in range(4):
                        nc.tensor.matmul(
                            pmm,
                            wo_sb[:, ci, mt * 128:(mt + 1) * 128],
                            a_sb[:, ci, qc * 512:(qc + 1) * 512],
                            start=(ci == 0), stop=(ci == 3),
                        )
                    ot = ostage.tile([128, 512], F32, tag="ot", bufs=3, name="ot")
                    nc.vector.tensor_copy(ot, pmm)
                    nc.sync.dma_start(
                        out=oT_d[mt * 128:(mt + 1) * 128, qc * 512:(qc + 1) * 512],
                        in_=ot,
                    )
    nc.finalize()
    return nc


_NC = None


def _get_nc():
    global _NC
    if _NC is None:
        _NC = build_nc()
    return _NC


def _shard_inputs(x, Wq, bq, Wk, bk, Wv, bv, Wo, bo):
    bf = ml_dtypes.bfloat16
    x = np.asarray(x, np.float32)
    in_maps = []
    wqT = np.ascontiguousarray(np.asarray(Wq, np.float32).T).astype(bf)  # [C, C] = [c_in, c_out]
    wkT = np.ascontiguousarray(np.asarray(Wk, np.float32).T).astype(bf)
    wvT = np.ascontiguousarray(np.asarray(Wv, np.float32).T).astype(bf)
    woT = np.ascontiguousarray(np.asarray(Wo, np.float32).T).astype(bf)  # [c_in, c_out]
    xT = [np.ascontiguousarray(x[b].T).astype(bf) for b in range(B)]
    for c in range(8):
        b, g = c % B, c // B
        sl = slice(g * CS, (g + 1) * CS)
        in_maps.append({
            "xT": xT[b],
            "wqT": np.ascontiguousarray(wqT[:, sl]),
            "wkT": np.ascontiguousarray(wkT[:, sl]),
            "wvT": np.ascontiguousarray(wvT[:, sl]),
            "woT": np.ascontiguousarray(woT[sl, :]),
            "bq": np.ascontiguousarray(np.asarray(bq, np.float32)[sl]).reshape(CS, 1),
            "bk": np.ascontiguousarray(np.asarray(bk, np.float32)[sl]).reshape(CS, 1),
        })
    return in_maps


def run_sharded(inputs, **kwargs):
    """Run the SPMD kernel; returns the BassKernelResults."""
    nc = _get_nc()
    in_maps = _shard_inputs(**inputs)
    return run_bass_kernel_spmd(nc, in_maps, core_ids=list(range(8)), **kwargs)


def assemble(results, Wv_bias, Wo, bo):
    bo_eff = (np.asarray(bo, np.float32)
              + np.asarray(Wo, np.float32) @ np.asarray(Wv_bias, np.float32))
    out = np.empty((B, T, C), np.float32)
    for b in range(B):
        acc = results[b]["oT"].astype(np.float32) + results[b + B]["oT"].astype(np.float32)
        out[b] = acc.T + bo_eff[None, :]
    return out


def kernel(**inputs):
    res = run_sharded(inputs)
    return assemble(res.results, inputs["bv"], inputs["Wo"], inputs["bo"])
